# revision 1
# baseline (speedup 1.0000x reference)
"""Trainium2 Bass kernel for nn_MixvMFGrad (mixture-of-vMF log-density gradient).

Math (per row s of the batch, d=512, K=64 components):
    dots  = s @ mus^T                       [K]
    t_k   = delta_k + kappa_k * dots_k      (delta = coef - max coef, folded on host)
    e     = exp(t)                          (no row-max shift needed: |t| <= ~35 by
                                             construction for this input distribution)
    g     = e @ mus                         [d]
    q     = g . s  = sum_k e_k * dots_k
    n2    = |g|^2  = e^T G e,   G = mus @ mus^T   (host precomputed)
    out   = (g - q s) / sqrt(n2)

Device layout: rows sharded 8 ways (data-parallel); per core the batch is
processed in supertiles of 512 rows.  dots are computed transposed
([K, rows] = wk-chunks^T @ s^T-chunks, s^T built with PE transposes) so the
per-component constants (delta exp-bias, kappa scaling folded into wk) are
per-partition; q and n2 are reduced over k with tiny per-subtile matmuls
against [-1/kappa | 1], landing directly in per-partition [128,1] layout;
1/sqrt(n2) is a DVE bit-trick + 2 Newton steps (ACT Rsqrt is banned for
accuracy, and exp/ln would thrash ACT table sets); the tangent projection is
one fused scalar_tensor_tensor per subtile plus a per-partition scale.
MM_MODE=fp32r runs matmuls in the PE's fast reduced-precision fp32 mode
(~2.6e-4 rel err vs ~1e-6 for fp32, ~2x faster).
"""

import os
from contextlib import ExitStack

import numpy as np

import concourse.bass as bass
import concourse.tile as tile
from concourse import bacc
from concourse import mybir
from concourse.bass_utils import run_bass_kernel_spmd

N_CORES = 8
BS = 200000
D = 512
K = 64
ROWS_PER_CORE = BS // N_CORES  # 25000
ST_ROWS = 512                  # rows per supertile
PAD_ROWS = 25088               # 49 supertiles of 512
F32 = mybir.dt.float32
F32R = mybir.dt.float32r

# "fp32" (full precision, rel err ~5e-6, 762us) or "fp32r" (PE fast fp32 mode,
# rel err ~2.6e-4, 552us). Default to full precision: its error sits below the
# reference's own fp32 rounding envelope, so it cannot fail a correctness gate.
MM_MODE = os.environ.get("MIXVMF_MM_MODE", "fp32")

LAST_RESULT = None  # test.py reads exec_time_ns off this


DT = F32R if MM_MODE == "fp32r" else F32


def _f32(ap):
    """View a DT AP as plain fp32 (for elementwise engines)."""
    if MM_MODE == "fp32r":
        return ap.bitcast(F32)
    return ap


def build_nc(rows=PAD_ROWS):
    assert rows % ST_ROWS == 0
    n_st = rows // ST_ROWS
    nc = bacc.Bacc("TRN2", target_bir_lowering=False)

    s_d = nc.dram_tensor("s", [rows, D], DT, kind="ExternalInput")
    out_d = nc.dram_tensor("out", [rows, D], F32, kind="ExternalOutput")
    wk_d = nc.dram_tensor("wk", [128, 4, K], DT, kind="ExternalInput")
    musr_d = nc.dram_tensor("musr", [K, D], DT, kind="ExternalInput")
    gmat_d = nc.dram_tensor("gmat", [K, K], DT, kind="ExternalInput")
    delta_d = nc.dram_tensor("delta", [K, 1], F32, kind="ExternalInput")
    iv1_d = nc.dram_tensor("iv1", [K, 2], DT, kind="ExternalInput")
    ident_d = nc.dram_tensor("ident", [128, 128], DT, kind="ExternalInput")

    AF = mybir.ActivationFunctionType
    OP = mybir.AluOpType

    # [rows, D] viewed as [p, q, d] per 512-row supertile (q = 128-row subtile)
    s_v = s_d[:].rearrange("(t q p) d -> t p q d", p=128, q=4)
    o_v = out_d[:].rearrange("(t q p) d -> t p q d", p=128, q=4)

    with tile.TileContext(nc) as tc, ExitStack() as ctx:
        consts = ctx.enter_context(tc.tile_pool(name="consts", bufs=1))
        in_pool = ctx.enter_context(tc.tile_pool(name="in_pool", bufs=4))
        out_pool = ctx.enter_context(tc.tile_pool(name="out_pool", bufs=4))
        sT_pool = ctx.enter_context(tc.tile_pool(name="sT_pool", bufs=3))
        small = ctx.enter_context(tc.tile_pool(name="small", bufs=4))
        ps_T = ctx.enter_context(tc.tile_pool(name="ps_T", bufs=2, space="PSUM"))
        ps_AC = ctx.enter_context(tc.tile_pool(name="ps_AC", bufs=3, space="PSUM"))
        ps_g = ctx.enter_context(tc.tile_pool(name="ps_g", bufs=2, space="PSUM"))
        ps_row = ctx.enter_context(tc.tile_pool(name="ps_row", bufs=1, space="PSUM"))

        wk_sb = consts.tile([128, 4, K], DT)
        nc.sync.dma_start(out=wk_sb, in_=wk_d[:])
        musr_sb = consts.tile([K, D], DT)
        nc.sync.dma_start(out=musr_sb, in_=musr_d[:])
        gmat_sb = consts.tile([K, K], DT)
        nc.sync.dma_start(out=gmat_sb, in_=gmat_d[:])
        delta_sb = consts.tile([K, 1], F32)
        nc.sync.dma_start(out=delta_sb, in_=delta_d[:])
        iv1_sb = consts.tile([K, 2], DT)
        nc.sync.dma_start(out=iv1_sb, in_=iv1_d[:])
        ident_sb = consts.tile([128, 128], DT)
        nc.sync.dma_start(out=ident_sb, in_=ident_d[:])

        for st in range(n_st):
            s_t = in_pool.tile([128, 4, D], DT, tag="s")
            nc.sync.dma_start(out=s_t, in_=s_v[st])
            o_t = out_pool.tile([128, 4, D], F32, tag="o")

            # s^T chunks: 16 PE transposes + 4 PSUM->SBUF copies
            sT_sb = sT_pool.tile([128, 4, D], DT, tag="sT")
            for c in range(4):
                sT_ps = ps_T.tile([128, D], DT, tag="T")
                for q in range(4):
                    nc.tensor.transpose(
                        sT_ps[:, 128 * q:128 * (q + 1)],
                        s_t[:, q, 128 * c:128 * (c + 1)],
                        ident_sb,
                    )
                nc.scalar.copy(sT_sb[:, c, :], sT_ps)

            # A = dots2^T [K, 512] accumulated over 4 d-chunks
            A = ps_AC.tile([K, D], F32, tag="AC")
            for c in range(4):
                nc.tensor.matmul(
                    A, wk_sb[:, c, :], sT_sb[:, c, :],
                    start=(c == 0), stop=(c == 3),
                )

            e_t = small.tile([K, D], DT, tag="e")
            nc.scalar.activation(e_t, A, AF.Exp, bias=delta_sb)

            # h^T = G @ e
            C = ps_AC.tile([K, D], F32, tag="AC")
            nc.tensor.matmul(C, gmat_sb, e_t, start=True, stop=True)

            u_t = small.tile([K, D], DT, tag="u")
            nc.vector.tensor_mul(u_t, _f32(e_t), A)       # e * dots2
            p_t = small.tile([K, D], DT, tag="p")
            nc.vector.tensor_mul(p_t, _f32(e_t), C)       # e * (G e)

            # per-subtile reductions over k, landing directly in per-partition
            # layout (fp32r matmuls need N>=2, so rhs = [-1/kappa | ones] and
            # one junk column per matmul): col 4j = -q_j, col 4j+3 = n2_j
            qn_ps = ps_row.tile([128, 16], F32, tag="row")
            for j in range(4):
                nc.tensor.matmul(
                    qn_ps[:, 4 * j:4 * j + 2],
                    u_t[:, 128 * j:128 * (j + 1)], iv1_sb,
                    start=True, stop=True)
                nc.tensor.matmul(
                    qn_ps[:, 4 * j + 2:4 * j + 4],
                    p_t[:, 128 * j:128 * (j + 1)], iv1_sb,
                    start=True, stop=True)
            qr_sb = small.tile([128, 16], F32, tag="qr")
            nc.vector.tensor_copy(qr_sb, qn_ps)
            qr_v = qr_sb.rearrange("p (j c) -> p j c", c=4)

            # r = rsqrt(n2) on DVE: bit-trick seed + 2 Newton steps (batched
            # over the 4 subtiles; [128,4] tiles, all ops tiny)
            nr = small.tile([128, 20], F32, tag="nr")
            x = qr_v[:, :, 3]
            xi = x.bitcast(mybir.dt.int32)
            y0i = nr[:, 0:4].bitcast(mybir.dt.int32)
            nc.vector.tensor_scalar(
                out=nr[:, 16:20].bitcast(mybir.dt.int32), in0=xi,
                scalar1=1, scalar2=None, op0=OP.arith_shift_right)
            nc.vector.tensor_scalar(
                out=y0i, in0=nr[:, 16:20].bitcast(mybir.dt.int32),
                scalar1=-1, scalar2=0x5F3759DF, op0=OP.mult, op1=OP.add)
            y = nr[:, 0:4]
            for it in range(2):
                h1 = nr[:, 4 + 4 * it:8 + 4 * it]
                nc.vector.tensor_mul(h1, x, y)        # x*y
                nc.vector.tensor_mul(h1, h1, y)       # x*y^2
                nc.vector.tensor_scalar(
                    out=h1, in0=h1, scalar1=-0.5, scalar2=1.5,
                    op0=OP.mult, op1=OP.add)          # 1.5 - 0.5*x*y^2
                yn = nr[:, 12:16] if it == 0 else nr[:, 0:4]
                nc.vector.tensor_mul(yn, h1, y)
                y = yn
            for j in range(4):
                g_ps = ps_g.tile([128, D], F32, tag="g")
                nc.tensor.matmul(
                    g_ps, e_t[:, 128 * j:128 * (j + 1)], musr_sb,
                    start=True, stop=True,
                )
                # o = (s * (-q)) + g = g - q s
                nc.vector.scalar_tensor_tensor(
                    out=o_t[:, j, :], in0=_f32(s_t[:, j, :]),
                    scalar=qr_sb[:, 4 * j:4 * j + 1], in1=g_ps,
                    op0=OP.mult, op1=OP.add,
                )
                # o *= r (alternate DVE/ACT to balance engine load)
                if j % 2 == 0:
                    nc.vector.tensor_scalar_mul(
                        o_t[:, j, :], o_t[:, j, :], y[:, j:j + 1])
                else:
                    nc.scalar.mul(o_t[:, j, :], o_t[:, j, :], y[:, j:j + 1])

            nc.scalar.dma_start(out=o_v[st], in_=o_t)

    nc.finalize()
    return nc


def host_prep(alphas, mus, kappas):
    """Host-side fp64 precompute of the tiny per-component constants."""
    a = np.asarray(alphas, np.float64)
    m = np.asarray(mus, np.float64)
    k = np.asarray(kappas, np.float64)
    d = m.shape[1]
    nu = 0.5 * d - 1.0
    z = k / nu
    sq = np.sqrt(1.0 + z * z)
    eta = sq + np.log(z) - np.log1p(sq)
    t = 1.0 / sq
    u1 = (3.0 * t - 5.0 * t ** 3) / 24.0
    u2 = (81.0 * t ** 2 - 462.0 * t ** 4 + 385.0 * t ** 6) / 1152.0
    log_iv = (nu * eta - 0.5 * np.log(2.0 * np.pi * nu)
              - 0.25 * np.log1p(z * z) + np.log1p(u1 / nu + u2 / (nu * nu)))
    logC = d * (-0.5 * np.log(2.0 * np.pi)) + nu * np.log(k) - log_iv
    coef = np.log(a) + np.log(k) + logC
    delta = (coef - coef.max()).astype(np.float32).reshape(K, 1)

    musk = (k[:, None] * m)                    # kappa_k * mus_k
    # wk[p, c, j] = musk[j, 128c + p]
    wk = np.ascontiguousarray(
        musk.reshape(K, 4, 128).transpose(2, 1, 0).astype(np.float32))
    musr = np.asarray(mus, np.float32)
    gmat = (m @ m.T).astype(np.float32)
    iv1 = np.stack([-1.0 / k, np.ones(K)], axis=1).astype(np.float32)  # [-1/kappa | 1]
    ident = np.eye(128, dtype=np.float32)
    return dict(wk=wk, musr=musr, gmat=gmat, delta=delta, iv1=iv1, ident=ident)


_NC_CACHE = {}


def kernel(s, alphas, mus, kappas):
    global LAST_RESULT
    s = np.ascontiguousarray(np.asarray(s, np.float32))
    consts = host_prep(alphas, mus, kappas)

    rows = PAD_ROWS
    if rows not in _NC_CACHE:
        _NC_CACHE[rows] = build_nc(rows)
    nc = _NC_CACHE[rows]

    in_maps = []
    for c in range(N_CORES):
        shard = s[c * ROWS_PER_CORE:(c + 1) * ROWS_PER_CORE]
        pad = rows - shard.shape[0]
        if pad:
            shard = np.concatenate([shard, shard[:pad]], axis=0)
        in_maps.append({"s": np.ascontiguousarray(shard), **consts})

    res = run_bass_kernel_spmd(
        nc, in_maps, list(range(N_CORES)),
        trace=bool(os.environ.get("MIXVMF_TRACE")),
    )
    LAST_RESULT = res
    out = np.concatenate(
        [res.results[c]["out"][:ROWS_PER_CORE] for c in range(N_CORES)], axis=0)
    return out

